# revision 1
# baseline (speedup 1.0000x reference)
"""ChebNet GNN forward on trn2: 8-way node-sharded dense stages on device.

The per-layer dense work (4-way Chebyshev matmul combine + bias + activation)
runs as an SPMD Bass kernel on 8 NeuronCores, feature-major, node-sharded.
Sparse propagations (CSR segment sums) + BN stats run on host (the GpSimd
engine needed for indirect gather / collectives is unavailable here).
"""
import os
import sys
import types
import contextlib
import ctypes
import functools

sys.path.insert(0, '/opt/trn_rl_repo')
import numpy as np

N = 50000
E = 800000
H = 128
K = 4
P = 8
SH = 6250            # nodes per core
SHP = 6656           # padded to 13*512
NT = SHP // 512      # moving tiles per core
EPS_BN = np.float32(1e-5)
EPS_NORM = np.float32(1e-12)

HW_NS = []           # exec_time_ns per traced device call (test harness reads)

_cache = {}


def _install_ntff_hook():
    if "antenv" in sys.modules or True:
        try:
            import antenv
        except Exception:
            return
    so_path = "/opt/axon/libaxon_pjrt.so"
    if not os.path.exists(so_path):
        return
    lib = ctypes.CDLL(so_path)
    if not hasattr(lib, "axon_start_nrt_profile"):
        return
    lib.axon_start_nrt_profile.argtypes = [ctypes.POINTER(ctypes.c_int64),
                                           ctypes.c_size_t]
    lib.axon_start_nrt_profile.restype = ctypes.c_int64
    lib.axon_stop_nrt_profile.argtypes = [ctypes.c_char_p]
    lib.axon_stop_nrt_profile.restype = ctypes.c_int64

    @contextlib.contextmanager
    def _h(output_dir, device_ids):
        import jax
        jax.devices()
        if device_ids:
            ids = (ctypes.c_int64 * len(device_ids))(*device_ids)
            rc = lib.axon_start_nrt_profile(ids, len(device_ids))
        else:
            rc = lib.axon_start_nrt_profile(None, 0)
        if rc != 0:
            raise RuntimeError(f"axon_start_nrt_profile rc={rc}")
        try:
            yield
        finally:
            lib.axon_stop_nrt_profile(str(output_dir).encode())

    mod = types.ModuleType("antenv.axon_hooks")
    _hook = _h

    def set_axon_ntff_profile_hook(h):
        pass

    def get_axon_ntff_profile_hook():
        return _hook

    mod.set_axon_ntff_profile_hook = set_axon_ntff_profile_hook
    mod.get_axon_ntff_profile_hook = get_axon_ntff_profile_hook
    sys.modules["antenv.axon_hooks"] = mod
    antenv.axon_hooks = mod


def _build():
    from concourse import bacc, tile, mybir
    f32 = mybir.dt.float32
    nc = bacc.Bacc(None, num_devices=P)
    yts = [nc.dram_tensor(f"y{k}", [128, SHP], f32, kind="ExternalInput")
           for k in range(K)]
    wt = nc.dram_tensor("w", [K, 128, 128], f32, kind="ExternalInput")
    bt = nc.dram_tensor("b", [128, 1], f32, kind="ExternalInput")
    st = nc.dram_tensor("s", [128, 1], f32, kind="ExternalInput")
    out = nc.dram_tensor("h", [128, SHP], f32, kind="ExternalOutput")

    with tile.TileContext(nc) as tc:
        with tc.tile_pool(name="big", bufs=1) as big, \
             tc.tile_pool(name="pool", bufs=3) as pool, \
             tc.tile_pool(name="psum", bufs=2, space="PSUM") as psum:
            wsb = big.tile([128, K, 128], f32)
            bsb = big.tile([128, 1], f32)
            ssb = big.tile([128, 1], f32)
            nc.sync.dma_start(wsb[:], wt[:].rearrange("k p q -> p k q"))
            nc.sync.dma_start(bsb[:], bt[:])
            nc.sync.dma_start(ssb[:], st[:])
            for t in range(NT):
                acc = psum.tile([128, 512], f32)
                sl = slice(t * 512, (t + 1) * 512)
                yt0 = pool.tile([128, 512], f32)
                yt1 = pool.tile([128, 512], f32)
                yt2 = pool.tile([128, 512], f32)
                yt3 = pool.tile([128, 512], f32)
                yti = [yt0, yt1, yt2, yt3]
                for k in range(K):
                    nc.sync.dma_start(yti[k][:], yts[k][:, sl])
                for k in range(K):
                    nc.tensor.matmul(acc[:], wsb[:, k, :], yti[k][:],
                                     start=(k == 0), stop=(k == K - 1))
                hb = pool.tile([128, 512], f32)
                nc.vector.tensor_scalar_add(hb[:], acc[:], bsb[:, 0:1])
                ho = pool.tile([128, 512], f32)
                nc.vector.scalar_tensor_tensor(
                    ho[:], hb[:], ssb[:, 0:1], hb[:],
                    mybir.AluOpType.mult, mybir.AluOpType.max)
                nc.sync.dma_start(out[:, sl], ho[:])
    nc.compile()
    return nc


def _dev_layer(yTs, Wk, b, slope):
    """yTs: list of 4 arrays [128, N] f32. Returns h_pre [128, N] f32."""
    from concourse.bass_utils import run_bass_kernel_spmd
    if "nc" not in _cache:
        if os.environ.get("BASS_KERNEL_TRACE"):
            _install_ntff_hook()
        _cache["nc"] = _build()
    nc = _cache["nc"]
    in_maps = []
    for c in range(P):
        m = {}
        for k in range(K):
            sh = np.zeros((128, SHP), np.float32)
            sh[:, :SH] = yTs[k][:, c * SH:(c + 1) * SH]
            m[f"y{k}"] = sh
        m["w"] = Wk
        m["b"] = b.reshape(128, 1).astype(np.float32)
        m["s"] = np.full((128, 1), slope, np.float32)
        in_maps.append(m)
    trace = bool(os.environ.get("BASS_KERNEL_TRACE"))
    res = None
    for attempt in range(3):
        try:
            res = run_bass_kernel_spmd(nc, in_maps, core_ids=list(range(P)),
                                       trace=trace)
            break
        except Exception:
            if attempt == 2:
                raise
    if trace and res.exec_time_ns:
        HW_NS.append(res.exec_time_ns)
    return np.concatenate([res.results[c]["h"][:, :SH] for c in range(P)], 1)


def _pad_w(W):
    """W [K, Din, H] -> [K, 128, 128] zero-padded."""
    Wp = np.zeros((K, 128, 128), np.float32)
    Wp[:, :W.shape[1], :W.shape[2]] = W
    return Wp


def kernel(x, edge_index, W1, b1, W2, b2, W3, b3, W4, b4,
           g1, be1, g2, be2, g3, be3, Wm, bm):
    from scipy.sparse import csr_matrix
    x = np.asarray(x, np.float32)
    ei = np.asarray(edge_index)
    src, dst = ei[0].astype(np.int64), ei[1].astype(np.int64)
    deg = np.bincount(src, minlength=N).astype(np.float32)
    dinv = np.where(deg > 0, 1.0 / np.sqrt(np.maximum(deg, 1.0)), 0.0) \
             .astype(np.float32)
    w = (-dinv[src] * dinv[dst]).astype(np.float32)
    A = csr_matrix((w, (dst, src)), shape=(N, N), dtype=np.float32)

    def cheb_ys(h):
        t0 = h
        t1 = A @ h
        t2 = 2.0 * (A @ t1) - t0
        t3 = 2.0 * (A @ t2) - t1
        return [np.asarray(t, np.float32) for t in (t0, t1, t2, t3)]

    def to_T(ys):
        out = []
        for y in ys:
            yT = np.zeros((128, N), np.float32)
            yT[:y.shape[1], :] = y.T
            out.append(yT)
        return out

    def bn(h, g, be):
        m = h.mean(0, dtype=np.float32)
        v = np.square(h - m).mean(0, dtype=np.float32)
        return ((h - m) / np.sqrt(v + EPS_BN) * g + be).astype(np.float32)

    h = x
    for (W, b, slope, gg, bb) in [(W1, b1, 0.01, g1, be1),
                                  (W2, b2, 0.01, g2, be2),
                                  (W3, b3, 0.0, g3, be3)]:
        hp = _dev_layer(to_T(cheb_ys(h)), _pad_w(np.asarray(W, np.float32)),
                        np.pad(np.asarray(b, np.float32), (0, 128 - len(b))),
                        slope).T[:, :H]
        h = bn(hp, np.asarray(gg, np.float32), np.asarray(bb, np.float32))

    hp = _dev_layer(to_T(cheb_ys(h)), _pad_w(np.asarray(W4, np.float32)),
                    np.asarray(b4, np.float32), 1.0).T[:, :H]
    r = np.maximum(np.linalg.norm(hp, axis=1, keepdims=True), EPS_NORM)
    hn = (hp / r).astype(np.float32)
    return (hn @ np.asarray(Wm, np.float32) +
            np.asarray(bm, np.float32)).astype(np.float32)



# revision 10
# speedup vs baseline: 1.8299x; 1.8299x over previous
"""ChebNet GNN forward on trn2: 8-way node-sharded dense stages on device.

Per-layer dense work (4-way Chebyshev matmul combine + bias + activation)
runs as SPMD Bass kernels on 8 NeuronCores, feature-major, node-sharded,
in fp16 (inputs/outputs) with f32 PSUM accumulation. Sparse propagations
(CSR segment sums) + BN stats run on host (no GpSimd indirect gather /
collectives available here).

Layout tricks vs the f32 baseline:
- L1 input is only 3 features wide: all 4 Chebyshev terms pack into a
  13-partition moving tensor (12 data rows + ones row for the bias), so
  layer 1 is one matmul per tile and ~3% of the old traffic.
- L2-L4 inputs are k-interleaved per column tile so each tile is one
  contiguous [128, 4*512] fp16 DMA.
- Bias is applied by the PE via an extra ones-row matmul into the same
  PSUM accumulation group; the only DVE work per tile is the activation.
- L4 folds the final L2-normalize + projection: the device emits
  z = Wm^T h4 [3, n] and sumsq [1, n]; host does z/sqrt(s) + bm.
"""
import os
import sys
import types
import contextlib
import ctypes

sys.path.insert(0, '/opt/trn_rl_repo')
import numpy as np

N = 50000
E = 800000
H = 128
K = 4
P = 8
SH = 6250            # nodes per core
TILE = 512
TILES = []
_c = 0
while _c < SH:
    TILES.append((_c, min(TILE, SH - _c)))
    _c += TILES[-1][1]
EPS_BN = np.float32(1e-5)
EPS_NORM = np.float32(1e-12)

HW_NS = []           # exec_time_ns per traced device call (test harness reads)

_cache = {}


def _install_ntff_hook():
    if "antenv" in sys.modules or True:
        try:
            import antenv
        except Exception:
            return
    so_path = "/opt/axon/libaxon_pjrt.so"
    if not os.path.exists(so_path):
        return
    lib = ctypes.CDLL(so_path)
    if not hasattr(lib, "axon_start_nrt_profile"):
        return
    lib.axon_start_nrt_profile.argtypes = [ctypes.POINTER(ctypes.c_int64),
                                           ctypes.c_size_t]
    lib.axon_start_nrt_profile.restype = ctypes.c_int64
    lib.axon_stop_nrt_profile.argtypes = [ctypes.c_char_p]
    lib.axon_stop_nrt_profile.restype = ctypes.c_int64

    @contextlib.contextmanager
    def _h(output_dir, device_ids):
        import jax
        jax.devices()
        if device_ids:
            ids = (ctypes.c_int64 * len(device_ids))(*device_ids)
            rc = lib.axon_start_nrt_profile(ids, len(device_ids))
        else:
            rc = lib.axon_start_nrt_profile(None, 0)
        if rc != 0:
            raise RuntimeError(f"axon_start_nrt_profile rc={rc}")
        try:
            yield
        finally:
            lib.axon_stop_nrt_profile(str(output_dir).encode())

    mod = types.ModuleType("antenv.axon_hooks")
    _hook = _h

    def set_axon_ntff_profile_hook(h):
        pass

    def get_axon_ntff_profile_hook():
        return _hook

    mod.set_axon_ntff_profile_hook = set_axon_ntff_profile_hook
    mod.get_axon_ntff_profile_hook = get_axon_ntff_profile_hook
    sys.modules["antenv.axon_hooks"] = mod
    antenv.axon_hooks = mod


def _build_l1():
    from concourse import bacc, tile, mybir
    f16, f32 = mybir.dt.float16, mybir.dt.float32
    nc = bacc.Bacc(None, num_devices=P)
    ys = nc.dram_tensor("ys", [13, SH], f16, kind="ExternalInput")
    ws = nc.dram_tensor("ws", [13, 128], f16, kind="ExternalInput")
    al = nc.dram_tensor("al", [128, 1], f32, kind="ExternalInput")
    g = nc.dram_tensor("g", [128, SH], f16, kind="ExternalOutput")
    with tile.TileContext(nc) as tc:
        with tc.tile_pool(name="big", bufs=1) as big, \
             tc.tile_pool(name="pool", bufs=4) as pool, \
             tc.tile_pool(name="psum", bufs=4, space="PSUM") as psum:
            wsb = big.tile([13, 128], f16)
            asb = big.tile([128, 1], f32)
            ysb = big.tile([13, SH], f16)
            nc.sync.dma_start(wsb[:], ws[:])
            nc.sync.dma_start(asb[:], al[:])
            nc.sync.dma_start(ysb[:], ys[:])
            for (c0, w) in TILES:
                acc = psum.tile([128, TILE], f32)
                nc.tensor.matmul(acc[:, :w], wsb[:], ysb[:, c0:c0 + w],
                                 start=True, stop=True)
                ho = pool.tile([128, TILE], f16)
                nc.scalar.activation(ho[:, :w], acc[:, :w],
                                     mybir.ActivationFunctionType.Lrelu,
                                     alpha=asb[:, 0:1])
                nc.sync.dma_start(g[:, c0:c0 + w], ho[:, :w])
    nc.compile()
    return nc


CHUNKS = [TILES[i:i + 2] for i in range(0, len(TILES), 2)]


def _build_l23():
    from concourse import bacc, tile, mybir
    f16, f32 = mybir.dt.float16, mybir.dt.float32
    nc = bacc.Bacc(None, num_devices=P)
    yc = nc.dram_tensor("yc", [128, 4 * SH], f16, kind="ExternalInput")
    wt = nc.dram_tensor("w", [128, 4 * 128], f16, kind="ExternalInput")
    bt = nc.dram_tensor("b", [128, 1], f32, kind="ExternalInput")
    al = nc.dram_tensor("al", [128, 1], f32, kind="ExternalInput")
    g = nc.dram_tensor("g", [128, SH], f16, kind="ExternalOutput")
    with tile.TileContext(nc) as tc:
        with tc.tile_pool(name="big", bufs=1) as big, \
             tc.tile_pool(name="pool", bufs=3) as pool, \
             tc.tile_pool(name="out", bufs=3) as outp, \
             tc.tile_pool(name="psum", bufs=4, space="PSUM") as psum:
            wsb = big.tile([128, 4 * 128], f16)
            bsb = big.tile([128, 1], f32)
            asb = big.tile([128, 1], f32)
            nc.sync.dma_start(wsb[:], wt[:])
            nc.sync.dma_start(bsb[:], bt[:])
            nc.sync.dma_start(asb[:], al[:])
            for chunk in CHUNKS:
                cb = chunk[0][0]
                cw = sum(w for (_, w) in chunk)
                yt = pool.tile([128, 2 * 4 * TILE], f16)
                nc.sync.dma_start(yt[:, :4 * cw], yc[:, 4 * cb:4 * (cb + cw)])
                ho = outp.tile([128, 2 * TILE], f16)
                for (c0, w) in chunk:
                    o = 4 * (c0 - cb)
                    acc = psum.tile([128, TILE], f32)
                    for k in range(K):
                        nc.tensor.matmul(
                            acc[:, :w], wsb[:, k * 128:(k + 1) * 128],
                            yt[:, o + k * w:o + (k + 1) * w],
                            start=(k == 0), stop=(k == K - 1))
                    nc.scalar.activation(ho[:, c0 - cb:c0 - cb + w],
                                         acc[:, :w],
                                         mybir.ActivationFunctionType.Lrelu,
                                         bias=bsb[:, 0:1], alpha=asb[:, 0:1])
                nc.sync.dma_start(g[:, cb:cb + cw], ho[:, :cw])
    nc.compile()
    return nc


def _build_l4():
    from concourse import bacc, tile, mybir
    f16, f32 = mybir.dt.float16, mybir.dt.float32
    nc = bacc.Bacc(None, num_devices=P)
    yc = nc.dram_tensor("yc", [128, 4 * SH], f16, kind="ExternalInput")
    wt = nc.dram_tensor("w", [128, 4 * 128], f16, kind="ExternalInput")
    bt = nc.dram_tensor("b", [128, 1], f32, kind="ExternalInput")
    wm = nc.dram_tensor("wm", [128, 3], f16, kind="ExternalInput")
    zs = nc.dram_tensor("zs", [4, SH], f16, kind="ExternalOutput")
    with tile.TileContext(nc) as tc:
        with tc.tile_pool(name="big", bufs=1) as big, \
             tc.tile_pool(name="pool", bufs=3) as pool, \
             tc.tile_pool(name="mid", bufs=3) as midp, \
             tc.tile_pool(name="psum", bufs=4, space="PSUM") as psum, \
             tc.tile_pool(name="psz", bufs=2, space="PSUM") as psz, \
             tc.tile_pool(name="pss", bufs=2, space="PSUM") as pss:
            wsb = big.tile([128, 4 * 128], f16)
            bsb = big.tile([128, 1], f32)
            wmb = big.tile([128, 3], f16)
            ones = big.tile([128, 1], f16)
            zbig = big.tile([3, SH], f16)
            sbig = big.tile([1, SH], f16)
            nc.sync.dma_start(wsb[:], wt[:])
            nc.sync.dma_start(bsb[:], bt[:])
            nc.sync.dma_start(wmb[:], wm[:])
            nc.vector.memset(ones[:], 1.0)
            for chunk in CHUNKS:
                cb = chunk[0][0]
                cw = sum(w for (_, w) in chunk)
                yt = pool.tile([128, 2 * 4 * TILE], f16)
                nc.sync.dma_start(yt[:, :4 * cw], yc[:, 4 * cb:4 * (cb + cw)])
                for (c0, w) in chunk:
                    o = 4 * (c0 - cb)
                    acc = psum.tile([128, TILE], f32)
                    for k in range(K):
                        nc.tensor.matmul(
                            acc[:, :w], wsb[:, k * 128:(k + 1) * 128],
                            yt[:, o + k * w:o + (k + 1) * w],
                            start=(k == 0), stop=(k == K - 1))
                    hb = midp.tile([128, TILE], f16, tag="hb")
                    nc.vector.tensor_scalar_add(hb[:, :w], acc[:, :w],
                                                bsb[:, 0:1])
                    sq = midp.tile([128, TILE], f16, tag="sq")
                    nc.vector.tensor_mul(sq[:, :w], hb[:, :w], hb[:, :w])
                    z = psz.tile([3, TILE], f32)
                    nc.tensor.matmul(z[:, :w], wmb[:], hb[:, :w],
                                     start=True, stop=True)
                    s1 = pss.tile([1, TILE], f32)
                    nc.tensor.matmul(s1[:, :w], ones[:], sq[:, :w],
                                     start=True, stop=True)
                    nc.vector.tensor_copy(zbig[:, c0:c0 + w], z[:, :w])
                    nc.vector.tensor_copy(sbig[:, c0:c0 + w], s1[:, :w])
            nc.sync.dma_start(zs[0:3, :], zbig[:])
            nc.sync.dma_start(zs[3:4, :], sbig[:])
    nc.compile()
    return nc


def _run(nc, in_maps):
    from concourse.bass_utils import run_bass_kernel_spmd
    trace = bool(os.environ.get("BASS_KERNEL_TRACE"))
    res = None
    for attempt in range(3):
        try:
            res = run_bass_kernel_spmd(nc, in_maps, core_ids=list(range(P)),
                                       trace=trace)
            break
        except Exception:
            if attempt == 2:
                raise
    if trace and res.exec_time_ns:
        HW_NS.append(res.exec_time_ns)
    return res.results


def kernel(x, edge_index, W1, b1, W2, b2, W3, b3, W4, b4,
           g1, be1, g2, be2, g3, be3, Wm, bm):
    from scipy.sparse import csr_matrix
    x = np.asarray(x, np.float32)
    ei = np.asarray(edge_index)
    src, dst = ei[0].astype(np.int64), ei[1].astype(np.int64)
    deg = np.bincount(src, minlength=N).astype(np.float32)
    dinv = np.where(deg > 0, 1.0 / np.sqrt(np.maximum(deg, 1.0)), 0.0) \
             .astype(np.float32)
    w = (-dinv[src] * dinv[dst]).astype(np.float32)
    A = csr_matrix((w, (dst, src)), shape=(N, N), dtype=np.float32)

    if "l1" not in _cache:
        if os.environ.get("BASS_KERNEL_TRACE"):
            _install_ntff_hook()
        _cache["l1"] = _build_l1()
        _cache["l23"] = _build_l23()
        _cache["l4"] = _build_l4()

    def cheb_ys(h):
        t0 = h
        t1 = A @ h
        t2 = 2.0 * (A @ t1) - t0
        t3 = 2.0 * (A @ t2) - t1
        return [np.asarray(t, np.float32) for t in (t0, t1, t2, t3)]

    def bn(h, g, be):
        m = h.mean(0, dtype=np.float32)
        v = np.square(h - m).mean(0, dtype=np.float32)
        return ((h - m) / np.sqrt(v + EPS_BN) * g + be).astype(np.float32)

    def pack_yc(Ts):
        Tt = [np.ascontiguousarray(t.T).astype(np.float16) for t in Ts]
        maps = []
        for c in range(P):
            b0 = c * SH
            ycm = np.empty((128, 4 * SH), np.float16)
            for (c0, w_) in TILES:
                for k in range(K):
                    ycm[:, 4 * c0 + k * w_: 4 * c0 + (k + 1) * w_] = \
                        Tt[k][:, b0 + c0: b0 + c0 + w_]
            maps.append(ycm)
        return maps

    # ---- Layer 1: [N,3] features, packed into 13 partitions ----
    ys = cheb_ys(x)
    ysT = np.ones((13, N), np.float16)
    for k in range(K):
        ysT[3 * k:3 * k + 3, :] = ys[k].T
    ws = np.zeros((13, 128), np.float32)
    for k in range(K):
        ws[3 * k:3 * k + 3, :] = np.asarray(W1, np.float32)[k]
    ws[12, :] = np.asarray(b1, np.float32)
    ws16 = ws.astype(np.float16)
    al = np.full((128, 1), 0.01, np.float32)
    in_maps = [{"ys": np.ascontiguousarray(ysT[:, c * SH:(c + 1) * SH]),
                "ws": ws16, "al": al} for c in range(P)]
    res = _run(_cache["l1"], in_maps)
    g = np.concatenate([res[c]["g"] for c in range(P)], 1)
    h = bn(g.T.astype(np.float32), np.asarray(g1, np.float32),
           np.asarray(be1, np.float32))

    # ---- Layers 2,3 ----
    for (W, b, slope, gam, bet) in [(W2, b2, 0.01, g2, be2),
                                    (W3, b3, 0.0, g3, be3)]:
        ycs = pack_yc(cheb_ys(h))
        Wf = np.asarray(W, np.float32)
        wst = np.concatenate([Wf[k] for k in range(K)], 1).astype(np.float16)
        brow = np.asarray(b, np.float32).reshape(128, 1)
        alv = np.full((128, 1), slope, np.float32)
        in_maps = [{"yc": ycs[c], "w": wst, "b": brow, "al": alv}
                   for c in range(P)]
        res = _run(_cache["l23"], in_maps)
        g = np.concatenate([res[c]["g"] for c in range(P)], 1)
        h = bn(g.T.astype(np.float32), np.asarray(gam, np.float32),
               np.asarray(bet, np.float32))

    # ---- Layer 4 + normalize + projection ----
    ycs = pack_yc(cheb_ys(h))
    Wf = np.asarray(W4, np.float32)
    wst = np.concatenate([Wf[k] for k in range(K)], 1).astype(np.float16)
    brow = np.asarray(b4, np.float32).reshape(128, 1)
    wm16 = np.asarray(Wm, np.float32).astype(np.float16)
    in_maps = [{"yc": ycs[c], "w": wst, "b": brow, "wm": wm16}
               for c in range(P)]
    res = _run(_cache["l4"], in_maps)
    zs = np.concatenate([res[c]["zs"] for c in range(P)], 1)
    z = zs[:3].astype(np.float32)
    s = zs[3].astype(np.float32)
    r = np.maximum(np.sqrt(s), EPS_NORM)
    out = (z / r).T + np.asarray(bm, np.float32)
    return out.astype(np.float32)
